# revision 30
# baseline (speedup 1.0000x reference)
"""PoolHiddenNet-style GNN message passing kernel for 8 Trainium2 cores.

Math (per group s of S=32, P=64 peds, uniform groups):
  rel[i,j]  = obs[j] - obs[i]                         (P^2, 16)
  emb       = rel @ W_sp + b_sp                       (P^2, 512)
  x_a       = tw * emb          tw[n, t*64+k] = twq[n, t*2+k%2]
  x1        = relu(bn([x_a, h1] @ W1 + b1))           (P^2, 512)
  x2        = relu(bn(x1 @ W2 + b2))                  (P^2, 1024)
  out       = max over j                              (P, 1024)

Key transforms used here:
  * b1/b2 cancel inside train-mode BN (bias shifts the mean equally).
  * tw*emb @ W1a == z @ C with z[n, q*16+r] = twq[n,q]*rel[n,r] and
    C[q*16+r, d] = sum_{f: q(f)=q} W_sp[r,f] W1a[f,d]  (K 576 -> 256).
    b_sp contributes twq @ Cb with Cb[q,d] = sum_{f:q(f)=q} b_sp[f] W1a[f,d];
    that 16-row k-chunk is only emitted when Cb != 0 (b_sp nonzero).
  * The h1 @ W1b term is constant over i (h1[i*P+j] = h[j]), so it is
    NOT part of the big matmul: hb[d, j] = (h @ W1b)^T is a 64-column
    matmul per group, and the PSUM->SBUF evict of x1 runs on DVE as a
    tensor_tensor add of hb broadcast over i.  This drops the third
    k-chunk from the x1 matmul (16K fewer PE rows per group).
  * BN2 apply is monotone (gamma*rsqrt > 0), so max-pool first, then
    apply BN+relu on the pooled (P, 1024) values only.
  * x2 is never evicted: max-pool stage 1+2 (j 64->32->16, TT-max) run
    on GpSimd/Pool reading PSUM directly and writing fp16; the final
    16->1 reduce runs on DVE.  Sum-of-squares (BN2 var) drains each
    PSUM tile via ACT Square+accum_out or DVE tensor_tensor_reduce
    (split set by the ACT_SQ_PX knob); mean2 comes from colsum(x1n)
    (BN1-apply accum_out) via thin matmuls on the PE.
  * The issue order interleaves x1(g+1) and x2(g) dch-by-dch so the
    strict-FIFO ACT/DVE queues never head-of-line-block the shared
    PSUM pool; BN2-finalize runs per-dch, one slot behind its dch.
  * z operands: zT[q*16+r, n] = twqT[q, n] * relT[r, n] built by
    tensor_tensor mults; on DVE (2x mode, fp16) for the prologue groups
    (startup critical path), on Pool for steady-state groups.  z DMAs
    issue at the iteration top, mults at the bottom.
  * Everything runs feature-on-partition (transposed activations); the
    output leaves via PE transposes + BN2 apply on GpSimd + a
    block-stitching DMA access pattern.

Sharding: data-parallel over S; core c handles groups 4c..4c+3.
"""

import os
import numpy as np

S, P = 32, 64
PP = P * P                  # 4096
OBS, EMB, HDIM = 8, 64, 64
D1, D2 = 512, 1024
NCORES = 8
G = S // NCORES             # 4 groups per core
EPS = 1e-5

F16 = np.float16
# matmul/operand dtype for the main chain ("f16" or "f32")
MM_DTYPE = os.environ.get("KERNEL_MM_DTYPE", "f16")
# of the 4 x1 dchs per group, how many evict via DVE tensor_tensor
# (fused h-add); the rest evict via ACT Copy + Pool h-add
EVICT_DVE_DCH = int(os.environ.get("KERNEL_EVICT_DVE_DCH", "0"))
# x1 evict for the non-DVE dchs: "act" (ACT Copy) or "dma"
# (gpsimd-initiated casting DMA, PSUM f32 -> SBUF fp16)
EVICT_MODE = os.environ.get("KERNEL_EVICT_MODE", "act")
# BN1 apply granularity: number of ACT ops per dch (1, 2 or 4)
APPLY_CHUNKS = int(os.environ.get("KERNEL_APPLY_CHUNKS", "2"))
# PSUM tile width: 0 -> four [128,2,512] tiles per dch (2 banks each),
# 1 -> two [128,4,512] tiles per dch (4 banks each, fewer/wider drains;
# measured worse: the 2-slot ring serializes the x1/x2 interleave)
PSUM_WIDE = int(os.environ.get("KERNEL_PSUM_WIDE", "0"))
# groups whose z_mult runs on DVE (prologue); the rest run on Pool
Z_DVE_GROUPS = int(os.environ.get("KERNEL_Z_DVE_GROUPS", "2"))
# 1: h-term via the k3 matmul k-chunk (baseline); 0: h-term added by the
# evict (ACT evict + Pool h-add), k3 chunk dropped from the PE
H_K3 = int(os.environ.get("KERNEL_H_K3", "0"))

_PROG_CACHE = {}
LAST_RESULTS = None


def _np_mm_dtype():
    return np.float32 if MM_DTYPE == "f32" else F16


def build_program(has_cb=False):
    """Build (and compile) the per-core Bass program. Returns nc."""
    import concourse.bacc as bacc
    import concourse.mybir as mybir
    import concourse.tile as tile
    from concourse import masks

    f32 = mybir.dt.float32
    DT = mybir.dt.float32 if MM_DTYPE == "f32" else mybir.dt.float16
    AF = mybir.ActivationFunctionType
    ALU = mybir.AluOpType

    nc = bacc.Bacc("TRN2", target_bir_lowering=False, debug=False)

    # ---- DRAM I/O ----
    d_reld = nc.dram_tensor("reld", [G * 16, PP], DT, kind="ExternalInput")
    d_twqT = nc.dram_tensor("twqT", [16, G, PP], DT, kind="ExternalInput")
    d_hT = nc.dram_tensor("hT", [HDIM, G, P], DT, kind="ExternalInput")
    d_C = nc.dram_tensor("Csb", [128, 2, D1], DT, kind="ExternalInput")
    d_W1b = nc.dram_tensor("W1b", [HDIM, D1], DT, kind="ExternalInput")
    d_Cb = (nc.dram_tensor("Cb", [16, D1], DT, kind="ExternalInput")
            if has_cb else None)
    d_W2 = nc.dram_tensor("W2sb", [128, 4, D2], DT, kind="ExternalInput")
    d_g1 = nc.dram_tensor("g1c", [128, 4], f32, kind="ExternalInput")
    d_be1 = nc.dram_tensor("be1c", [128, 4], f32, kind="ExternalInput")
    d_g2 = nc.dram_tensor("g2c", [128, 8], f32, kind="ExternalInput")
    d_be2 = nc.dram_tensor("be2c", [128, 8], f32, kind="ExternalInput")
    d_out = nc.dram_tensor("out", [G * P, D2], f32, kind="ExternalOutput")

    HF = PP // 2

    with tile.TileContext(nc) as tc:
        with (
            tc.tile_pool(name="singles", bufs=1) as singles,
            tc.tile_pool(name="work", bufs=2) as work,
            tc.tile_pool(name="stat", bufs=2) as stat,
            tc.tile_pool(name="psmm", bufs=(2 if PSUM_WIDE else 4),
                         space="PSUM") as psmm,
            tc.tile_pool(name="dscr", bufs=2, space="DRAM") as dscr,
        ):
            H_K3_ = H_K3
            Csb = singles.tile([128, 2, D1], DT)
            if H_K3_:
                CbWs = singles.tile([(16 if has_cb else 0) + HDIM, D1], DT)
                W1bs = Cbs = hTs = None
            else:
                W1bs = singles.tile([HDIM, D1], DT)
                Cbs = singles.tile([16, D1], DT) if has_cb else None
                hTs = singles.tile([HDIM, G, P], DT)
                CbWs = None
            W2sb = singles.tile([128, 4, D2], DT)
            g1c = singles.tile([128, 4], f32)
            be1c = singles.tile([128, 4], f32)
            g2c = singles.tile([128, 8], f32)
            be2c = singles.tile([128, 8], f32)
            eps_t = singles.tile([128, 1], f32)
            ident = singles.tile([128, 128], f32)

            n_groups = int(os.environ.get("KERNEL_GROUPS", G))

            # k3 chunk (H_K3 mode): [twq(16, when cb); h(64)] rows
            K3R = (16 if has_cb else 0) + HDIM
            K3H = 16 if has_cb else 0

            def z_dma(g, cb_eng=None, a_eng=None):
                """z operand DMA-expansion (+ the k3 rows in H_K3 mode /
                the twq k-chunk rows when Cb is present).  a_eng picks the
                DMA queue for the A operands (the prologue spreads them on
                the idle Pool queue so the first z_mult unblocks sooner)."""
                cb_eng = cb_eng or nc.sync
                a_eng = a_eng or nc.sync
                zT = work.tile([128, 2, PP], DT, tag="zT")
                if H_K3:
                    ktw = work.tile([K3R, PP], DT, tag="ktw")
                elif has_cb:
                    ktw = work.tile([16, PP], DT, tag="ktw")
                else:
                    ktw = None
                abs_ = []
                for h in range(2):
                    B_h = work.tile([128, HF], DT, tag="bsb")
                    nc.sync.dma_start(
                        out=B_h[:],
                        in_=d_reld.ap()[None, g * 16:g * 16 + 16,
                                        h * HF:(h + 1) * HF]
                        .broadcast_to((8, 16, HF)))
                    As = []
                    for kc in range(2):
                        A_h = work.tile([128, HF], DT, tag="asb")
                        a_eng.dma_start(
                            out=A_h[:],
                            in_=d_twqT.ap()[8 * kc:8 * kc + 8, g, None,
                                            h * HF:(h + 1) * HF]
                            .broadcast_to((8, 16, HF)))
                        As.append(A_h)
                    abs_.append((B_h, As))
                    if has_cb:
                        cb_eng.dma_start(
                            out=ktw[0:16, h * HF:(h + 1) * HF],
                            in_=d_twqT.ap()[:, g, h * HF:(h + 1) * HF])
                    if H_K3:
                        cb_eng.dma_start(
                            out=ktw[K3H:, h * HF:(h + 1) * HF]
                            .rearrange("p (i j) -> p i j", j=P),
                            in_=d_hT.ap()[:, g, None, :]
                            .broadcast_to((HDIM, HF // P, P)))
                return zT, ktw, abs_

            def z_mult(zk, eng=None):
                """zT[q*16+r, n] = twqT[q, n] * relT[r, n], in quarter-
                chunks so the first x1 matmul columns unblock as soon as
                the corresponding DMA quarter lands.  eng: DVE (2x fp16,
                prologue) or Pool (steady state)."""
                eng = eng or nc.gpsimd
                zT, _ktw, abs_ = zk
                Q = HF // 2
                for h in range(2):
                    B_h, As = abs_[h]
                    for q in range(2):
                        for kc in range(2):
                            eng.tensor_tensor(
                                out=zT[:, kc, h * HF + q * Q:
                                       h * HF + (q + 1) * Q],
                                in0=As[kc][:, q * Q:(q + 1) * Q],
                                in1=B_h[:, q * Q:(q + 1) * Q], op=ALU.mult)

            def hb_make(g):
                """hb[d(128 part), dch, j] = (h @ W1b)^T for group g: four
                64-column matmuls into one psum slot, one ACT evict."""
                pxh = psmm.tile([128, 2, 512], f32, tag="mm")
                hv = pxh[:, 0, 0:256].rearrange("p (a b) -> p a b", a=4)
                for dch in range(4):
                    nc.tensor.matmul(hv[:, dch, :],
                                     W1bs[:, dch * 128:(dch + 1) * 128],
                                     hTs[:, g, :], start=True, stop=True)
                hbs = stat.tile([128, 4, P], f32, tag="hbs")
                nc.scalar.activation(out=hbs[:], in_=hv, func=AF.Copy)
                return hbs

            def x1_dch(zT, ktw, hbs, x1, s1np, dch, apply_prio=-150):
                """One dch (128 feats) of x1: 4 psum tiles of matmuls,
                ACT evict to SBUF fp16 (+ Pool h-add when the k3 chunk is
                dropped), DVE bn_stats, BN1 chain, fused ACT apply+relu
                (accum->s1n)."""
                d0 = dch * 128
                stats1 = stat.tile([128, 8, 6], f32, tag="stats1")
                QW = 4 if PSUM_WIDE else 2
                CW = QW * 512
                NI = CW // P
                for t in range(PP // CW):
                    px = psmm.tile([128, QW, 512], f32, tag="mm")
                    last = 2 if (H_K3 or has_cb) else 1
                    # kc-outer so consecutive matmuls share the lhsT
                    for q in range(QW):
                        n0 = t * CW + q * 512
                        nc.tensor.matmul(px[:, q, :],
                                         Csb[:, 0, d0:d0 + 128],
                                         zT[:, 0, n0:n0 + 512],
                                         start=True, stop=False)
                    for q in range(QW):
                        n0 = t * CW + q * 512
                        nc.tensor.matmul(px[:, q, :],
                                         Csb[:, 1, d0:d0 + 128],
                                         zT[:, 1, n0:n0 + 512],
                                         start=False, stop=last == 1)
                    if H_K3 or has_cb:
                        lhs = CbWs if H_K3 else Cbs
                        for q in range(QW):
                            n0 = t * CW + q * 512
                            nc.tensor.matmul(px[:, q, :],
                                             lhs[:, d0:d0 + 128],
                                             ktw[:, n0:n0 + 512],
                                             start=False, stop=True)
                    xs = x1[:, dch, t * CW:(t + 1) * CW]
                    xv = xs.rearrange("p (i j) -> p i j", j=P)
                    if dch < EVICT_DVE_DCH:
                        # DVE evict (+ fused h-add when the k3 chunk is
                        # dropped): only DVE/ACT may read PSUM, so the
                        # evict placement balances the two queues
                        if H_K3:
                            nc.vector.tensor_copy(
                                xs, px[:].rearrange("p a b -> p (a b)"))
                        else:
                            nc.vector.tensor_tensor(
                                out=xv,
                                in0=px[:].rearrange("p a b -> p (a b)")
                                .rearrange("p (i j) -> p i j", j=P),
                                in1=hbs[:, dch, :].unsqueeze(1)
                                .broadcast_to((128, NI, P)),
                                op=ALU.add)
                    else:
                        nc.scalar.activation(
                            out=xs,
                            in_=px[:].rearrange("p a b -> p (a b)"),
                            func=AF.Copy)
                        if not H_K3:
                            # h-term add on Pool, in place in SBUF
                            nc.gpsimd.tensor_tensor(
                                out=xv, in0=xv,
                                in1=hbs[:, dch, :].unsqueeze(1)
                                .broadcast_to((128, NI, P)),
                                op=ALU.add)
                for c in range(8):
                    nc.vector.bn_stats(
                        out=stats1[:, c, :],
                        in_=x1[:, dch, c * 512:(c + 1) * 512])
                mv1 = stat.tile([128, 2], f32, tag="mv1")
                nc.vector.bn_aggr(out=mv1[:], in_=stats1[:])
                std1 = stat.tile([128, 1], f32, tag="std1")
                gam1 = stat.tile([128, 1], f32, tag="gam1")
                bet1 = stat.tile([128, 1], f32, tag="bet1")
                nc.scalar.activation(out=std1[:], in_=mv1[:, 1:2],
                                     func=AF.Sqrt, bias=eps_t[:])
                nc.vector.reciprocal(out=std1[:], in_=std1[:])
                nc.vector.tensor_mul(gam1[:], g1c[:, dch:dch + 1], std1[:])
                nc.vector.tensor_mul(bet1[:], mv1[:, 0:1], gam1[:])
                nc.vector.tensor_sub(bet1[:], be1c[:, dch:dch + 1], bet1[:])
                # the apply's consumer (x2 of the NEXT group) is a full
                # iteration away; deprioritize it AND chunk it so the
                # px-draining Squares of the current x2 dch are never stuck
                # behind a 3.8us monolith on the strict-FIFO ACT queue
                with tc.high_priority(offset=apply_prio):
                    cw = PP // APPLY_CHUNKS
                    for c in range(APPLY_CHUNKS):
                        nc.scalar.activation(
                            out=x1[:, dch, c * cw:(c + 1) * cw],
                            in_=x1[:, dch, c * cw:(c + 1) * cw],
                            func=AF.Relu, bias=bet1[:], scale=gam1[:],
                            accum_out=s1np[:, dch, c, None])

            def mean2_start(s1np):
                """mean2 (transposed, [1, 1024]) via thin matmuls, then
                redistributed to [128, 8] through a DRAM scratch bounce."""
                s1n = stat.tile([128, 4], f32, tag="s1nr")
                nc.vector.reduce_sum(s1n[:], s1np[:],
                                     axis=mybir.AxisListType.X)
                s1nd = stat.tile([128, 4], DT, tag="s1nd")
                nc.vector.tensor_copy(s1nd[:], s1n[:])
                pm2 = psmm.tile([1, 2, 512], f32, tag="mm")
                for kc in range(4):
                    for hh in range(2):
                        nc.tensor.matmul(
                            pm2[:, hh, :], s1nd[:, kc:kc + 1],
                            W2sb[:, kc, hh * 512:(hh + 1) * 512],
                            start=(kc == 0), stop=(kc == 3))
                sum2 = stat.tile([1, 1024], f32, tag="sum2")
                nc.scalar.mul(out=sum2[:], in_=pm2[:].rearrange(
                    "p a b -> p (a b)"), mul=1.0 / PP)
                m2d = dscr.tile([1, 1024], f32, tag="m2d")
                nc.sync.dma_start(out=m2d[:], in_=sum2[:])
                mean2 = stat.tile([128, 8], f32, tag="mean2")
                nc.sync.dma_start(
                    out=mean2[:],
                    in_=m2d[:].rearrange("p (a b) -> (p b) a", a=8))
                return mean2

            def x2_core(g, x1, dch):
                """One dch (128 feats) of x2: 4 psum tiles of matmuls;
                per tile the max-tree stage1+2 run on Pool (PSUM -> fp16
                SBUF), the final 16->1 stage on DVE; sum-of-squares via
                ACT Square+accum or DVE ttr (ACT_SQ_PX split)."""
                d0 = dch * 128
                QW = 4 if PSUM_WIDE else 2
                CW = QW * 512
                NI = CW // P
                NPX = PP // CW
                ssqd = stat.tile([128, NPX], f32, tag="ssqd", bufs=4)
                pooled = stat.tile([128, P], f32, tag="pooled", bufs=8)
                for t in range(NPX):
                    px = psmm.tile([128, QW, 512], f32, tag="mm")
                    # kc-outer so consecutive matmuls share the lhsT
                    for kc in range(4):
                        for q in range(QW):
                            n0 = t * CW + q * 512
                            nc.tensor.matmul(
                                px[:, q, :], W2sb[:, kc, d0:d0 + 128],
                                x1[:, kc, n0:n0 + 512],
                                start=(kc == 0), stop=(kc == 3))
                    pxf = px[:].rearrange("p a b -> p (a b)")
                    # max over j straight from PSUM (DVE is the only
                    # engine that can max-reduce PSUM: GPSIMD has no PSUM
                    # access, TT ops allow only one PSUM operand)
                    nc.vector.reduce_max(
                        pooled[:, t * NI:(t + 1) * NI],
                        pxf.rearrange("p (i j) -> p i j", j=P),
                        axis=mybir.AxisListType.X)
                    # sum-of-squares: forced onto ACT (the only engine
                    # that can square-accumulate a PSUM tile; DVE ttr
                    # would need two PSUM reads, which BIR forbids)
                    sqj = work.tile([128, CW], DT, tag="sqj", bufs=4)
                    nc.scalar.activation(
                        out=sqj[:], in_=pxf, func=AF.Square,
                        accum_out=ssqd[:, t:t + 1])
                ssqt = stat.tile([128, 1], f32, tag="ssqt", bufs=4)
                nc.vector.reduce_sum(ssqt[:], ssqd[:],
                                     axis=mybir.AxisListType.X)
                return ssqt, pooled

            def x2_fin(g, mean2, gb, core_ctx, dch):
                """Per-dch BN2 finalize: var2 = sumsq/N - mean2^2; the
                gamma'/beta' land in the packed gb tile for out_half."""
                ssqt, _pooled = core_ctx
                m2 = mean2[:, dch:dch + 1]
                m2sq = stat.tile([128, 1], f32, tag="m2sq")
                nc.vector.tensor_mul(m2sq[:], m2, m2)
                var2 = stat.tile([128, 1], f32, tag="var2")
                nc.vector.scalar_tensor_tensor(
                    out=var2[:], in0=ssqt[:], scalar=1.0 / PP, in1=m2sq[:],
                    op0=ALU.mult, op1=ALU.subtract)
                std2 = stat.tile([128, 1], f32, tag="std2")
                gam2 = gb[:, 0, dch % 4, None]
                bet2 = gb[:, 1, dch % 4, None]
                nc.scalar.activation(out=std2[:], in_=var2[:],
                                     func=AF.Sqrt, bias=eps_t[:])
                nc.vector.reciprocal(out=std2[:], in_=std2[:])
                nc.vector.tensor_mul(gam2, g2c[:, dch:dch + 1], std2[:])
                nc.vector.tensor_mul(bet2, m2, gam2)
                nc.vector.tensor_sub(bet2, be2c[:, dch:dch + 1], bet2)

            def out_half(g, pools, gb, q4):
                """Batched PE transposes of the RAW pooled tiles (dep =
                max-tree only), then BN2 apply + relu in row-major layout
                on GpSimd: gam/bet get partition-broadcast via a DRAM
                bounce; one plain row-major DMA writes the half."""
                pst = psmm.tile([P, 4, 128], f32, tag="mm")
                for i, pl in enumerate(pools):
                    nc.tensor.transpose(pst[:, i, :], pl[:], ident[:])
                rows = stat.tile([P, 4, 128], f32, tag="rows")
                nc.vector.tensor_copy(rows[:], pst[:])
                gbd = dscr.tile([8, 128], f32, tag="gbd")
                nc.sync.dma_start(
                    out=gbd[:].rearrange("s f -> f s"),
                    in_=gb[:].rearrange("f s d -> f (s d)"))
                gbr = stat.tile([P, 8, 128], f32, tag="gbr")
                nc.sync.dma_start(
                    out=gbr[:],
                    in_=gbd[:].rearrange("s f -> (s f)")[None, :]
                    .broadcast_to((P, 1024)))
                nc.gpsimd.tensor_tensor(
                    out=rows[:], in0=rows[:], in1=gbr[:, 0:4, :],
                    op=ALU.mult)
                nc.gpsimd.tensor_tensor(
                    out=rows[:], in0=rows[:], in1=gbr[:, 4:8, :],
                    op=ALU.add)
                rws = rows[:].rearrange("p a b -> p (a b)")
                nc.gpsimd.tensor_relu(rws, rws)
                nc.sync.dma_start(
                    out=d_out.ap()[g * P:(g + 1) * P,
                                   q4 * 512:(q4 + 1) * 512],
                    in_=rws)

            def out_last(g, pools, gb):
                """Last half of the last group: feature-major GP apply +
                PE transposes (PE is drained by now), one contiguous DMA."""
                pq = stat.tile([128, 4, P], f32, tag="pqlast")
                for i, pl in enumerate(pools):
                    nc.gpsimd.tensor_scalar(
                        out=pq[:, i], in0=pl[:],
                        scalar1=gb[:, 0, i, None], scalar2=gb[:, 1, i, None],
                        op0=ALU.mult, op1=ALU.add)
                    nc.gpsimd.tensor_relu(pq[:, i], pq[:, i])
                pst = psmm.tile([P, 4, 128], f32, tag="mm")
                for i in range(4):
                    nc.tensor.transpose(pst[:, i, :], pq[:, i], ident[:])
                out_rows = stat.tile([P, 4, 128], f32, tag="rows")
                nc.vector.tensor_copy(out_rows[:], pst[:])
                nc.sync.dma_start(
                    out=d_out.ap()[g * P:(g + 1) * P, 512:1024],
                    in_=out_rows[:].rearrange("p a b -> p (a b)"))

            def x1_alloc():
                x1 = work.tile([128, 4, PP], DT, tag="x1")
                s1np = stat.tile([128, 4, APPLY_CHUNKS], f32, tag="s1n")
                return x1, s1np

            # prologue: first z-operands before the big weight loads (SP
            # queue); Csb rides the idle Pool queue (the ACT queue starts
            # with a 1.3us activation-table load); the rest go on ACT.
            nc.gpsimd.dma_start(out=Csb[:], in_=d_C.ap())
            for t_sb, t_dr in [
                (g1c, d_g1), (be1c, d_be1),
                (g2c, d_g2), (be2c, d_be2),
            ]:
                nc.scalar.dma_start(out=t_sb[:], in_=t_dr.ap())
            if H_K3:
                if has_cb:
                    nc.scalar.dma_start(out=CbWs[0:16, :], in_=d_Cb.ap())
                nc.scalar.dma_start(out=CbWs[K3H:, :], in_=d_W1b.ap())
            else:
                nc.scalar.dma_start(out=W1bs[:], in_=d_W1b.ap())
                if has_cb:
                    nc.scalar.dma_start(out=Cbs[:], in_=d_Cb.ap())
                nc.scalar.dma_start(out=hTs[:], in_=d_hT.ap())
            zks = [z_dma(0, a_eng=nc.gpsimd)]
            nc.vector.memset(eps_t[:], EPS)
            masks.make_identity(nc, ident[:])
            z_mult(zks[0], eng=nc.vector if Z_DVE_GROUPS > 0 else None)
            if n_groups > 1:
                zks.append(z_dma(1))
            # W2sb (1 MB, first needed by x2(0) / mean2 ~45us in) loads on
            # the SP queue behind the z expansions, keeping the ACT queue
            # clear for the first x1 evicts.
            nc.sync.dma_start(out=W2sb[:], in_=d_W2.ap())
            if n_groups > 1:
                z_mult(zks[1], eng=nc.vector if Z_DVE_GROUPS > 1 else None)

            # software pipeline: z(g+2) DMA prefetch at iteration top, its
            # mults at the bottom; x1(g+1) dchs interleaved with x2(g)
            # dchs; x2 finalize lags its core by one slot; mean2 at the
            # top of the iteration (k=1 of iteration 0).
            def hb_slot(g):
                return None if H_K3 else hb_make(g)

            hbs_l = [hb_slot(0)]
            if n_groups > 1:
                hbs_l.append(hb_slot(1))
            x1s = x1_alloc()
            for dch in range(4):
                # group 0: applies at normal priority — x2(g0) waits on
                # them directly, unlike the steady-state pipeline
                x1_dch(zks[0][0], zks[0][1], hbs_l[0], *x1s, dch,
                       apply_prio=0)
            for g in range(n_groups):
                x1, s1n = x1s
                mean2 = None
                if g + 1 < n_groups:
                    x1s = x1_alloc()
                cores = {}
                fctx = {}
                gb = stat.tile([128, 2, 4], f32, tag="gb")
                for k in range(4):
                    def x1_slot():
                        if g + 1 < n_groups:
                            x1_dch(zks[g + 1][0], zks[g + 1][1],
                                   hbs_l[g + 1], *x1s, k)
                    if g > 0:
                        x1_slot()
                    if k == 2:
                        gb_lo, gb = gb, stat.tile([128, 2, 4], f32,
                                                  tag="gb")
                    cores[2 * k] = x2_core(g, x1, 2 * k)
                    if k == 1:
                        mean2 = mean2_start(s1n)
                    if k == 1 and g + 2 < n_groups:
                        zks.append(z_dma(g + 2))
                        hbs_l.append(hb_slot(g + 2))
                    cores[2 * k + 1] = x2_core(g, x1, 2 * k + 1)
                    if g == 0:
                        x1_slot()
                    if k >= 1:
                        fin_gb = gb_lo if k == 2 else gb
                        for d in (2 * k - 2, 2 * k - 1):
                            fctx[d] = cores.pop(d)
                            x2_fin(g, mean2, fin_gb, fctx[d], d)
                        if k == 2:
                            out_half(g, [fctx[d][1] for d in range(4)],
                                     gb_lo, 0)
                for dch in range(6, 8):
                    fctx[dch] = cores.pop(dch)
                    x2_fin(g, mean2, gb, fctx[dch], dch)
                if g == n_groups - 1:
                    out_last(g, [fctx[d][1] for d in range(4, 8)], gb)
                else:
                    out_half(g, [fctx[d][1] for d in range(4, 8)], gb, 1)
                if g + 2 < n_groups:
                    z_mult(zks[g + 2],
                           eng=nc.vector if g + 2 < Z_DVE_GROUPS else None)

    nc.compile()
    return nc


def _host_prepare(inputs):
    """Slice/permute full inputs into 8 per-core in_maps (host-side).
    Returns (in_maps, has_cb)."""
    dtm = _np_mm_dtype()
    f32 = np.float32

    h_states = np.asarray(inputs["h_states"], f32)
    traj = np.asarray(inputs["traj"], f32)
    traj_weight = np.asarray(inputs["traj_weight"], f32)
    W_sp = np.asarray(inputs["W_sp"], f32)
    b_sp = np.asarray(inputs["b_sp"], f32)
    W1 = np.asarray(inputs["W1"], f32)
    g1 = np.asarray(inputs["g1"], f32)
    be1 = np.asarray(inputs["be1"], f32)
    W2 = np.asarray(inputs["W2"], f32)
    g2 = np.asarray(inputs["g2"], f32)
    be2 = np.asarray(inputs["be2"], f32)

    # obs: (S, P, 16) with feature index t*2+c
    obs = np.transpose(traj[:OBS], (1, 0, 2)).reshape(S, P, OBS * 2)
    h = h_states.reshape(S, P, HDIM)

    # relT[s, r, i*64+j] = obs[s, j, r] - obs[s, i, r]
    obsT = obs.transpose(0, 2, 1)                      # (S, 16, P)
    relT = (obsT[:, :, None, :] - obsT[:, :, :, None]).reshape(S, 16, PP)

    # C fold: q(f) = (f//64)*2 + f%2
    f_idx = np.arange(EMB * OBS)
    qof = (f_idx // EMB) * 2 + (f_idx % 2)
    W1a, W1b = W1[:D1], W1[D1:]
    C = np.zeros((256, D1), f32)
    Cb = np.zeros((16, D1), f32)
    for q in range(16):
        m = qof == q
        C[q * 16:(q + 1) * 16] = W_sp[:, m] @ W1a[m]
        Cb[q] = b_sp[m] @ W1a[m]
    has_cb = bool(np.any(Cb != 0.0))
    Csb = np.ascontiguousarray(C.reshape(2, 128, D1).transpose(1, 0, 2))
    W2sb = np.ascontiguousarray(W2.reshape(4, 128, D2).transpose(1, 0, 2))

    shared = {
        "Csb": Csb.astype(dtm),
        "W1b": W1b.astype(dtm),
        "W2sb": W2sb.astype(dtm),
        "g1c": np.ascontiguousarray(g1.reshape(4, 128).T),
        "be1c": np.ascontiguousarray(be1.reshape(4, 128).T),
        "g2c": np.ascontiguousarray(g2.reshape(8, 128).T),
        "be2c": np.ascontiguousarray(be2.reshape(8, 128).T),
    }
    if has_cb:
        shared["Cb"] = Cb.astype(dtm)

    in_maps = []
    for c in range(NCORES):
        sl = slice(c * G, (c + 1) * G)
        reld = np.ascontiguousarray(relT[sl].reshape(G * 16, PP))
        twqT = np.ascontiguousarray(
            traj_weight[sl].transpose(3, 2, 0, 1).reshape(16, G, PP))
        hT = np.ascontiguousarray(h[sl].transpose(2, 0, 1))     # (64,G,P)
        in_maps.append({
            "reld": reld.astype(dtm),
            "twqT": twqT.astype(dtm),
            "hT": hT.astype(dtm),
            **shared,
        })
    return in_maps, has_cb


def kernel(**inputs) -> np.ndarray:
    global LAST_RESULTS
    from concourse import bass_utils

    in_maps, has_cb = _host_prepare(inputs)
    key = ("prog", has_cb)
    if key not in _PROG_CACHE:
        _PROG_CACHE[key] = build_program(has_cb=has_cb)
    nc = _PROG_CACHE[key]

    trace = bool(int(os.environ.get("KERNEL_TRACE", "0")))
    res = bass_utils.run_bass_kernel_spmd(
        nc, in_maps, core_ids=list(range(NCORES)), trace=trace)
    LAST_RESULTS = res
    out = np.concatenate([res.results[c]["out"] for c in range(NCORES)], axis=0)
    return out.astype(np.float32)


# revision 31
# speedup vs baseline: 1.0020x; 1.0020x over previous
"""PoolHiddenNet-style GNN message passing kernel for 8 Trainium2 cores.

Math (per group s of S=32, P=64 peds, uniform groups):
  rel[i,j]  = obs[j] - obs[i]                         (P^2, 16)
  emb       = rel @ W_sp + b_sp                       (P^2, 512)
  x_a       = tw * emb          tw[n, t*64+k] = twq[n, t*2+k%2]
  x1        = relu(bn([x_a, h1] @ W1 + b1))           (P^2, 512)
  x2        = relu(bn(x1 @ W2 + b2))                  (P^2, 1024)
  out       = max over j                              (P, 1024)

Key transforms used here:
  * b1/b2 cancel inside train-mode BN (bias shifts the mean equally).
  * tw*emb @ W1a == z @ C with z[n, q*16+r] = twq[n,q]*rel[n,r] and
    C[q*16+r, d] = sum_{f: q(f)=q} W_sp[r,f] W1a[f,d]  (K 576 -> 256).
    b_sp contributes twq @ Cb with Cb[q,d] = sum_{f:q(f)=q} b_sp[f] W1a[f,d];
    that 16-row k-chunk is only emitted when Cb != 0 (b_sp nonzero).
  * The h1 @ W1b term is constant over i (h1[i*P+j] = h[j]), so it is
    NOT part of the big matmul: hb[d, j] = (h @ W1b)^T is a 64-column
    matmul per group, and the PSUM->SBUF evict of x1 runs on DVE as a
    tensor_tensor add of hb broadcast over i.  This drops the third
    k-chunk from the x1 matmul (16K fewer PE rows per group).
  * BN2 apply is monotone (gamma*rsqrt > 0), so max-pool first, then
    apply BN+relu on the pooled (P, 1024) values only.
  * x2 is never evicted: max-pool stage 1+2 (j 64->32->16, TT-max) run
    on GpSimd/Pool reading PSUM directly and writing fp16; the final
    16->1 reduce runs on DVE.  Sum-of-squares (BN2 var) drains each
    PSUM tile via ACT Square+accum_out or DVE tensor_tensor_reduce
    (split set by the ACT_SQ_PX knob); mean2 comes from colsum(x1n)
    (BN1-apply accum_out) via thin matmuls on the PE.
  * The issue order interleaves x1(g+1) and x2(g) dch-by-dch so the
    strict-FIFO ACT/DVE queues never head-of-line-block the shared
    PSUM pool; BN2-finalize runs per-dch, one slot behind its dch.
  * z operands: zT[q*16+r, n] = twqT[q, n] * relT[r, n] built by
    tensor_tensor mults; on DVE (2x mode, fp16) for the prologue groups
    (startup critical path), on Pool for steady-state groups.  z DMAs
    issue at the iteration top, mults at the bottom.
  * Everything runs feature-on-partition (transposed activations); the
    output leaves via PE transposes + BN2 apply on GpSimd + a
    block-stitching DMA access pattern.

Sharding: data-parallel over S; core c handles groups 4c..4c+3.
"""

import os
import numpy as np

S, P = 32, 64
PP = P * P                  # 4096
OBS, EMB, HDIM = 8, 64, 64
D1, D2 = 512, 1024
NCORES = 8
G = S // NCORES             # 4 groups per core
EPS = 1e-5

F16 = np.float16
# matmul/operand dtype for the main chain ("f16" or "f32")
MM_DTYPE = os.environ.get("KERNEL_MM_DTYPE", "f16")
# of the 4 x1 dchs per group, how many evict via DVE tensor_tensor
# (fused h-add); the rest evict via ACT Copy + Pool h-add
EVICT_DVE_DCH = int(os.environ.get("KERNEL_EVICT_DVE_DCH", "0"))
# x1 evict for the non-DVE dchs: "act" (ACT Copy) or "dma"
# (gpsimd-initiated casting DMA, PSUM f32 -> SBUF fp16)
EVICT_MODE = os.environ.get("KERNEL_EVICT_MODE", "act")
# BN1 apply granularity: number of ACT ops per dch (1, 2 or 4)
APPLY_CHUNKS = int(os.environ.get("KERNEL_APPLY_CHUNKS", "2"))
# PSUM tile width: 0 -> four [128,2,512] tiles per dch (2 banks each),
# 1 -> two [128,4,512] tiles per dch (4 banks each, fewer/wider drains;
# measured worse: the 2-slot ring serializes the x1/x2 interleave)
PSUM_WIDE = int(os.environ.get("KERNEL_PSUM_WIDE", "0"))
# groups whose z_mult runs on DVE (prologue); the rest run on Pool
Z_DVE_GROUPS = int(os.environ.get("KERNEL_Z_DVE_GROUPS", "2"))
# 1: h-term via the k3 matmul k-chunk (baseline); 0: h-term added by the
# evict (ACT evict + Pool h-add), k3 chunk dropped from the PE
H_K3 = int(os.environ.get("KERNEL_H_K3", "0"))

_PROG_CACHE = {}
LAST_RESULTS = None


def _np_mm_dtype():
    return np.float32 if MM_DTYPE == "f32" else F16


def build_program(has_cb=False):
    """Build (and compile) the per-core Bass program. Returns nc."""
    import concourse.bacc as bacc
    import concourse.mybir as mybir
    import concourse.tile as tile
    from concourse import masks

    f32 = mybir.dt.float32
    DT = mybir.dt.float32 if MM_DTYPE == "f32" else mybir.dt.float16
    AF = mybir.ActivationFunctionType
    ALU = mybir.AluOpType

    nc = bacc.Bacc("TRN2", target_bir_lowering=False, debug=False)

    # ---- DRAM I/O ----
    d_reld = nc.dram_tensor("reld", [G * 16, PP], DT, kind="ExternalInput")
    d_twqT = nc.dram_tensor("twqT", [16, G, PP], DT, kind="ExternalInput")
    d_hT = nc.dram_tensor("hT", [HDIM, G, P], DT, kind="ExternalInput")
    d_C = nc.dram_tensor("Csb", [128, 2, D1], DT, kind="ExternalInput")
    d_W1b = nc.dram_tensor("W1b", [HDIM, D1], DT, kind="ExternalInput")
    d_Cb = (nc.dram_tensor("Cb", [16, D1], DT, kind="ExternalInput")
            if has_cb else None)
    d_W2 = nc.dram_tensor("W2sb", [128, 4, D2], DT, kind="ExternalInput")
    d_g1 = nc.dram_tensor("g1c", [128, 4], f32, kind="ExternalInput")
    d_be1 = nc.dram_tensor("be1c", [128, 4], f32, kind="ExternalInput")
    d_g2 = nc.dram_tensor("g2c", [128, 8], f32, kind="ExternalInput")
    d_be2 = nc.dram_tensor("be2c", [128, 8], f32, kind="ExternalInput")
    d_out = nc.dram_tensor("out", [G * P, D2], f32, kind="ExternalOutput")

    HF = PP // 2

    with tile.TileContext(nc) as tc:
        with (
            tc.tile_pool(name="singles", bufs=1) as singles,
            tc.tile_pool(name="work", bufs=2) as work,
            tc.tile_pool(name="stat", bufs=2) as stat,
            tc.tile_pool(name="psmm", bufs=(2 if PSUM_WIDE else 4),
                         space="PSUM") as psmm,
            tc.tile_pool(name="dscr", bufs=2, space="DRAM") as dscr,
        ):
            H_K3_ = H_K3
            Csb = singles.tile([128, 2, D1], DT)
            if H_K3_:
                CbWs = singles.tile([(16 if has_cb else 0) + HDIM, D1], DT)
                W1bs = Cbs = hTs = None
            else:
                W1bs = singles.tile([HDIM, D1], DT)
                Cbs = singles.tile([16, D1], DT) if has_cb else None
                hTs = singles.tile([HDIM, G, P], DT)
                CbWs = None
            W2sb = singles.tile([128, 4, D2], DT)
            g1c = singles.tile([128, 4], f32)
            be1c = singles.tile([128, 4], f32)
            g2c = singles.tile([128, 8], f32)
            be2c = singles.tile([128, 8], f32)
            eps_t = singles.tile([128, 1], f32)
            ident = singles.tile([128, 128], f32)

            n_groups = int(os.environ.get("KERNEL_GROUPS", G))

            # k3 chunk (H_K3 mode): [twq(16, when cb); h(64)] rows
            K3R = (16 if has_cb else 0) + HDIM
            K3H = 16 if has_cb else 0

            def z_dma(g, cb_eng=None, a_eng=None):
                """z operand DMA-expansion (+ the k3 rows in H_K3 mode /
                the twq k-chunk rows when Cb is present).  a_eng picks the
                DMA queue for the A operands (the prologue spreads them on
                the idle Pool queue so the first z_mult unblocks sooner)."""
                cb_eng = cb_eng or nc.sync
                a_eng = a_eng or nc.sync
                zT = work.tile([128, 2, PP], DT, tag="zT")
                if H_K3:
                    ktw = work.tile([K3R, PP], DT, tag="ktw")
                elif has_cb:
                    ktw = work.tile([16, PP], DT, tag="ktw")
                else:
                    ktw = None
                abs_ = []
                for h in range(2):
                    B_h = work.tile([128, HF], DT, tag="bsb")
                    nc.sync.dma_start(
                        out=B_h[:],
                        in_=d_reld.ap()[None, g * 16:g * 16 + 16,
                                        h * HF:(h + 1) * HF]
                        .broadcast_to((8, 16, HF)))
                    As = []
                    for kc in range(2):
                        A_h = work.tile([128, HF], DT, tag="asb")
                        a_eng.dma_start(
                            out=A_h[:],
                            in_=d_twqT.ap()[8 * kc:8 * kc + 8, g, None,
                                            h * HF:(h + 1) * HF]
                            .broadcast_to((8, 16, HF)))
                        As.append(A_h)
                    abs_.append((B_h, As))
                    if has_cb:
                        cb_eng.dma_start(
                            out=ktw[0:16, h * HF:(h + 1) * HF],
                            in_=d_twqT.ap()[:, g, h * HF:(h + 1) * HF])
                    if H_K3:
                        cb_eng.dma_start(
                            out=ktw[K3H:, h * HF:(h + 1) * HF]
                            .rearrange("p (i j) -> p i j", j=P),
                            in_=d_hT.ap()[:, g, None, :]
                            .broadcast_to((HDIM, HF // P, P)))
                return zT, ktw, abs_

            def z_mult(zk, eng=None):
                """zT[q*16+r, n] = twqT[q, n] * relT[r, n], in quarter-
                chunks so the first x1 matmul columns unblock as soon as
                the corresponding DMA quarter lands.  eng: DVE (2x fp16,
                prologue) or Pool (steady state)."""
                eng = eng or nc.gpsimd
                zT, _ktw, abs_ = zk
                Q = HF // 2
                for h in range(2):
                    B_h, As = abs_[h]
                    for q in range(2):
                        for kc in range(2):
                            eng.tensor_tensor(
                                out=zT[:, kc, h * HF + q * Q:
                                       h * HF + (q + 1) * Q],
                                in0=As[kc][:, q * Q:(q + 1) * Q],
                                in1=B_h[:, q * Q:(q + 1) * Q], op=ALU.mult)

            def hb_make(g):
                """hb[d(128 part), dch, j] = (h @ W1b)^T for group g: four
                64-column matmuls into one psum slot, one ACT evict."""
                pxh = psmm.tile([128, 2, 512], f32, tag="mm")
                hv = pxh[:, 0, 0:256].rearrange("p (a b) -> p a b", a=4)
                for dch in range(4):
                    nc.tensor.matmul(hv[:, dch, :],
                                     W1bs[:, dch * 128:(dch + 1) * 128],
                                     hTs[:, g, :], start=True, stop=True)
                hbs = stat.tile([128, 4, P], f32, tag="hbs")
                nc.scalar.activation(out=hbs[:], in_=hv, func=AF.Copy)
                return hbs

            def x1_dch(zT, ktw, hbs, x1, s1np, dch, apply_prio=-150):
                """One dch (128 feats) of x1: 4 psum tiles of matmuls,
                ACT evict to SBUF fp16 (+ Pool h-add when the k3 chunk is
                dropped), DVE bn_stats, BN1 chain, fused ACT apply+relu
                (accum->s1n)."""
                d0 = dch * 128
                stats1 = stat.tile([128, 8, 6], f32, tag="stats1")
                QW = 4 if PSUM_WIDE else 2
                CW = QW * 512
                NI = CW // P
                for t in range(PP // CW):
                    px = psmm.tile([128, QW, 512], f32, tag="mm")
                    last = 2 if (H_K3 or has_cb) else 1
                    # kc-outer so consecutive matmuls share the lhsT
                    for q in range(QW):
                        n0 = t * CW + q * 512
                        nc.tensor.matmul(px[:, q, :],
                                         Csb[:, 0, d0:d0 + 128],
                                         zT[:, 0, n0:n0 + 512],
                                         start=True, stop=False)
                    for q in range(QW):
                        n0 = t * CW + q * 512
                        nc.tensor.matmul(px[:, q, :],
                                         Csb[:, 1, d0:d0 + 128],
                                         zT[:, 1, n0:n0 + 512],
                                         start=False, stop=last == 1)
                    if H_K3 or has_cb:
                        lhs = CbWs if H_K3 else Cbs
                        for q in range(QW):
                            n0 = t * CW + q * 512
                            nc.tensor.matmul(px[:, q, :],
                                             lhs[:, d0:d0 + 128],
                                             ktw[:, n0:n0 + 512],
                                             start=False, stop=True)
                    xs = x1[:, dch, t * CW:(t + 1) * CW]
                    xv = xs.rearrange("p (i j) -> p i j", j=P)
                    if dch < EVICT_DVE_DCH:
                        # DVE evict (+ fused h-add when the k3 chunk is
                        # dropped): only DVE/ACT may read PSUM, so the
                        # evict placement balances the two queues
                        if H_K3:
                            nc.vector.tensor_copy(
                                xs, px[:].rearrange("p a b -> p (a b)"))
                        else:
                            nc.vector.tensor_tensor(
                                out=xv,
                                in0=px[:].rearrange("p a b -> p (a b)")
                                .rearrange("p (i j) -> p i j", j=P),
                                in1=hbs[:, dch, :].unsqueeze(1)
                                .broadcast_to((128, NI, P)),
                                op=ALU.add)
                    else:
                        nc.scalar.activation(
                            out=xs,
                            in_=px[:].rearrange("p a b -> p (a b)"),
                            func=AF.Copy)
                        if not H_K3:
                            # h-term add on Pool, in place in SBUF
                            nc.gpsimd.tensor_tensor(
                                out=xv, in0=xv,
                                in1=hbs[:, dch, :].unsqueeze(1)
                                .broadcast_to((128, NI, P)),
                                op=ALU.add)
                for c in range(8):
                    nc.vector.bn_stats(
                        out=stats1[:, c, :],
                        in_=x1[:, dch, c * 512:(c + 1) * 512])
                mv1 = stat.tile([128, 2], f32, tag="mv1")
                nc.vector.bn_aggr(out=mv1[:], in_=stats1[:])
                std1 = stat.tile([128, 1], f32, tag="std1")
                gam1 = stat.tile([128, 1], f32, tag="gam1")
                bet1 = stat.tile([128, 1], f32, tag="bet1")
                nc.scalar.activation(out=std1[:], in_=mv1[:, 1:2],
                                     func=AF.Sqrt, bias=eps_t[:])
                nc.vector.reciprocal(out=std1[:], in_=std1[:])
                nc.vector.tensor_mul(gam1[:], g1c[:, dch:dch + 1], std1[:])
                nc.vector.tensor_mul(bet1[:], mv1[:, 0:1], gam1[:])
                nc.vector.tensor_sub(bet1[:], be1c[:, dch:dch + 1], bet1[:])
                # the apply's consumer (x2 of the NEXT group) is a full
                # iteration away; deprioritize it AND chunk it so the
                # px-draining Squares of the current x2 dch are never stuck
                # behind a 3.8us monolith on the strict-FIFO ACT queue
                with tc.high_priority(offset=apply_prio):
                    cw = PP // APPLY_CHUNKS
                    for c in range(APPLY_CHUNKS):
                        nc.scalar.activation(
                            out=x1[:, dch, c * cw:(c + 1) * cw],
                            in_=x1[:, dch, c * cw:(c + 1) * cw],
                            func=AF.Relu, bias=bet1[:], scale=gam1[:],
                            accum_out=s1np[:, dch, c, None])

            def mean2_start(s1np):
                """mean2 (transposed, [1, 1024]) via thin matmuls, then
                redistributed to [128, 8] through a DRAM scratch bounce."""
                s1n = stat.tile([128, 4], f32, tag="s1nr")
                nc.vector.reduce_sum(s1n[:], s1np[:],
                                     axis=mybir.AxisListType.X)
                s1nd = stat.tile([128, 4], DT, tag="s1nd")
                nc.vector.tensor_copy(s1nd[:], s1n[:])
                pm2 = psmm.tile([1, 2, 512], f32, tag="mm")
                for kc in range(4):
                    for hh in range(2):
                        nc.tensor.matmul(
                            pm2[:, hh, :], s1nd[:, kc:kc + 1],
                            W2sb[:, kc, hh * 512:(hh + 1) * 512],
                            start=(kc == 0), stop=(kc == 3))
                sum2 = stat.tile([1, 1024], f32, tag="sum2")
                nc.scalar.mul(out=sum2[:], in_=pm2[:].rearrange(
                    "p a b -> p (a b)"), mul=1.0 / PP)
                m2d = dscr.tile([1, 1024], f32, tag="m2d")
                nc.sync.dma_start(out=m2d[:], in_=sum2[:])
                mean2 = stat.tile([128, 8], f32, tag="mean2")
                nc.sync.dma_start(
                    out=mean2[:],
                    in_=m2d[:].rearrange("p (a b) -> (p b) a", a=8))
                return mean2

            def x2_core(g, x1, dch):
                """One dch (128 feats) of x2: 4 psum tiles of matmuls;
                per tile the max-tree stage1+2 run on Pool (PSUM -> fp16
                SBUF), the final 16->1 stage on DVE; sum-of-squares via
                ACT Square+accum or DVE ttr (ACT_SQ_PX split)."""
                d0 = dch * 128
                QW = 4 if PSUM_WIDE else 2
                CW = QW * 512
                NI = CW // P
                NPX = PP // CW
                ssqd = stat.tile([128, NPX], f32, tag="ssqd", bufs=4)
                pooled = stat.tile([128, P], f32, tag="pooled", bufs=8)
                for t in range(NPX):
                    px = psmm.tile([128, QW, 512], f32, tag="mm")
                    # kc-outer so consecutive matmuls share the lhsT
                    for kc in range(4):
                        for q in range(QW):
                            n0 = t * CW + q * 512
                            nc.tensor.matmul(
                                px[:, q, :], W2sb[:, kc, d0:d0 + 128],
                                x1[:, kc, n0:n0 + 512],
                                start=(kc == 0), stop=(kc == 3))
                    pxf = px[:].rearrange("p a b -> p (a b)")
                    # max over j straight from PSUM (DVE is the only
                    # engine that can max-reduce PSUM: GPSIMD has no PSUM
                    # access, TT ops allow only one PSUM operand)
                    nc.vector.reduce_max(
                        pooled[:, t * NI:(t + 1) * NI],
                        pxf.rearrange("p (i j) -> p i j", j=P),
                        axis=mybir.AxisListType.X)
                    # sum-of-squares: forced onto ACT (the only engine
                    # that can square-accumulate a PSUM tile; DVE ttr
                    # would need two PSUM reads, which BIR forbids)
                    sqj = work.tile([128, CW], DT, tag="sqj", bufs=4)
                    nc.scalar.activation(
                        out=sqj[:], in_=pxf, func=AF.Square,
                        accum_out=ssqd[:, t:t + 1])
                ssqt = stat.tile([128, 1], f32, tag="ssqt", bufs=4)
                nc.vector.reduce_sum(ssqt[:], ssqd[:],
                                     axis=mybir.AxisListType.X)
                return ssqt, pooled

            def x2_fin(g, mean2, gb, core_ctx, dch):
                """Per-dch BN2 finalize: var2 = sumsq/N - mean2^2; the
                gamma'/beta' land in the packed gb tile for out_half."""
                ssqt, _pooled = core_ctx
                m2 = mean2[:, dch:dch + 1]
                m2sq = stat.tile([128, 1], f32, tag="m2sq")
                nc.vector.tensor_mul(m2sq[:], m2, m2)
                var2 = stat.tile([128, 1], f32, tag="var2")
                nc.vector.scalar_tensor_tensor(
                    out=var2[:], in0=ssqt[:], scalar=1.0 / PP, in1=m2sq[:],
                    op0=ALU.mult, op1=ALU.subtract)
                std2 = stat.tile([128, 1], f32, tag="std2")
                gam2 = gb[:, 0, dch % 4, None]
                bet2 = gb[:, 1, dch % 4, None]
                nc.scalar.activation(out=std2[:], in_=var2[:],
                                     func=AF.Sqrt, bias=eps_t[:])
                nc.vector.reciprocal(out=std2[:], in_=std2[:])
                nc.vector.tensor_mul(gam2, g2c[:, dch:dch + 1], std2[:])
                nc.vector.tensor_mul(bet2, m2, gam2)
                nc.vector.tensor_sub(bet2, be2c[:, dch:dch + 1], bet2)

            def out_half(g, pools, gb, q4):
                """Batched PE transposes of the RAW pooled tiles (dep =
                max-tree only), then BN2 apply + relu in row-major layout
                on GpSimd: gam/bet get partition-broadcast via a DRAM
                bounce; one plain row-major DMA writes the half."""
                pst = psmm.tile([P, 4, 128], f32, tag="mm")
                for i, pl in enumerate(pools):
                    nc.tensor.transpose(pst[:, i, :], pl[:], ident[:])
                rows = stat.tile([P, 4, 128], f32, tag="rows")
                nc.vector.tensor_copy(rows[:], pst[:])
                gbd = dscr.tile([8, 128], f32, tag="gbd")
                nc.sync.dma_start(
                    out=gbd[:].rearrange("s f -> f s"),
                    in_=gb[:].rearrange("f s d -> f (s d)"))
                gbr = stat.tile([P, 8, 128], f32, tag="gbr")
                nc.sync.dma_start(
                    out=gbr[:],
                    in_=gbd[:].rearrange("s f -> (s f)")[None, :]
                    .broadcast_to((P, 1024)))
                nc.gpsimd.tensor_tensor(
                    out=rows[:], in0=rows[:], in1=gbr[:, 0:4, :],
                    op=ALU.mult)
                nc.gpsimd.tensor_tensor(
                    out=rows[:], in0=rows[:], in1=gbr[:, 4:8, :],
                    op=ALU.add)
                rws = rows[:].rearrange("p a b -> p (a b)")
                nc.gpsimd.tensor_relu(rws, rws)
                nc.sync.dma_start(
                    out=d_out.ap()[g * P:(g + 1) * P,
                                   q4 * 512:(q4 + 1) * 512],
                    in_=rws)

            def out_last(g, pools, gb):
                """Last half of the last group: feature-major GP apply +
                PE transposes (PE is drained by now), one contiguous DMA."""
                pq = stat.tile([128, 4, P], f32, tag="pqlast")
                for i, pl in enumerate(pools):
                    nc.gpsimd.tensor_scalar(
                        out=pq[:, i], in0=pl[:],
                        scalar1=gb[:, 0, i, None], scalar2=gb[:, 1, i, None],
                        op0=ALU.mult, op1=ALU.add)
                    nc.gpsimd.tensor_relu(pq[:, i], pq[:, i])
                pst = psmm.tile([P, 4, 128], f32, tag="mm")
                for i in range(4):
                    nc.tensor.transpose(pst[:, i, :], pq[:, i], ident[:])
                out_rows = stat.tile([P, 4, 128], f32, tag="rows")
                nc.vector.tensor_copy(out_rows[:], pst[:])
                nc.sync.dma_start(
                    out=d_out.ap()[g * P:(g + 1) * P, 512:1024],
                    in_=out_rows[:].rearrange("p a b -> p (a b)"))

            def x1_alloc():
                x1 = work.tile([128, 4, PP], DT, tag="x1")
                s1np = stat.tile([128, 4, APPLY_CHUNKS], f32, tag="s1n")
                return x1, s1np

            # prologue: first z-operands before the big weight loads (SP
            # queue); Csb rides the idle Pool queue (the ACT queue starts
            # with a 1.3us activation-table load); the rest go on ACT.
            nc.gpsimd.dma_start(out=Csb[:], in_=d_C.ap())
            for t_sb, t_dr in [
                (g1c, d_g1), (be1c, d_be1),
                (g2c, d_g2), (be2c, d_be2),
            ]:
                nc.scalar.dma_start(out=t_sb[:], in_=t_dr.ap())
            if H_K3:
                if has_cb:
                    nc.scalar.dma_start(out=CbWs[0:16, :], in_=d_Cb.ap())
                nc.scalar.dma_start(out=CbWs[K3H:, :], in_=d_W1b.ap())
            else:
                nc.scalar.dma_start(out=W1bs[:], in_=d_W1b.ap())
                if has_cb:
                    nc.scalar.dma_start(out=Cbs[:], in_=d_Cb.ap())
                nc.scalar.dma_start(out=hTs[:], in_=d_hT.ap())
            zks = [z_dma(0, a_eng=nc.gpsimd)]
            nc.vector.memset(eps_t[:], EPS)
            masks.make_identity(nc, ident[:])
            z_mult(zks[0], eng=nc.vector if Z_DVE_GROUPS > 0 else None)
            if n_groups > 1:
                zks.append(z_dma(1))
            # W2sb (1 MB, first needed by x2(0) / mean2 ~45us in) loads on
            # the SP queue behind the z expansions, keeping the ACT queue
            # clear for the first x1 evicts.
            nc.sync.dma_start(out=W2sb[:], in_=d_W2.ap())
            if n_groups > 1:
                z_mult(zks[1], eng=nc.vector if Z_DVE_GROUPS > 1 else None)

            # software pipeline: z(g+2) DMA prefetch at iteration top, its
            # mults at the bottom; x1(g+1) dchs interleaved with x2(g)
            # dchs; x2 finalize lags its core by one slot; mean2 at the
            # top of the iteration (k=1 of iteration 0).
            def hb_slot(g):
                return None if H_K3 else hb_make(g)

            hbs_l = [hb_slot(0)]
            if n_groups > 1:
                hbs_l.append(hb_slot(1))
            x1s = x1_alloc()
            for dch in range(4):
                # group 0: applies at normal priority — x2(g0) waits on
                # them directly, unlike the steady-state pipeline
                x1_dch(zks[0][0], zks[0][1], hbs_l[0], *x1s, dch,
                       apply_prio=int(os.environ.get("KERNEL_G0_PRIO", "0")))
            for g in range(n_groups):
                x1, s1n = x1s
                mean2 = None
                if g + 1 < n_groups:
                    x1s = x1_alloc()
                cores = {}
                fctx = {}
                gb = stat.tile([128, 2, 4], f32, tag="gb")
                for k in range(4):
                    def x1_slot():
                        if g + 1 < n_groups:
                            x1_dch(zks[g + 1][0], zks[g + 1][1],
                                   hbs_l[g + 1], *x1s, k)
                    if g > 0:
                        x1_slot()
                    if k == 2:
                        gb_lo, gb = gb, stat.tile([128, 2, 4], f32,
                                                  tag="gb")
                    cores[2 * k] = x2_core(g, x1, 2 * k)
                    if k == 1:
                        mean2 = mean2_start(s1n)
                    if k == 1 and g + 2 < n_groups:
                        zks.append(z_dma(g + 2))
                        hbs_l.append(hb_slot(g + 2))
                    cores[2 * k + 1] = x2_core(g, x1, 2 * k + 1)
                    if g == 0:
                        x1_slot()
                    if k >= 1:
                        fin_gb = gb_lo if k == 2 else gb
                        for d in (2 * k - 2, 2 * k - 1):
                            fctx[d] = cores.pop(d)
                            x2_fin(g, mean2, fin_gb, fctx[d], d)
                        if k == 2:
                            out_half(g, [fctx[d][1] for d in range(4)],
                                     gb_lo, 0)
                for dch in range(6, 8):
                    fctx[dch] = cores.pop(dch)
                    x2_fin(g, mean2, gb, fctx[dch], dch)
                if g == n_groups - 1:
                    out_last(g, [fctx[d][1] for d in range(4, 8)], gb)
                else:
                    out_half(g, [fctx[d][1] for d in range(4, 8)], gb, 1)
                if g + 2 < n_groups:
                    z_mult(zks[g + 2],
                           eng=nc.vector if g + 2 < Z_DVE_GROUPS else None)

    nc.compile()
    return nc


def _host_prepare(inputs):
    """Slice/permute full inputs into 8 per-core in_maps (host-side).
    Returns (in_maps, has_cb)."""
    dtm = _np_mm_dtype()
    f32 = np.float32

    h_states = np.asarray(inputs["h_states"], f32)
    traj = np.asarray(inputs["traj"], f32)
    traj_weight = np.asarray(inputs["traj_weight"], f32)
    W_sp = np.asarray(inputs["W_sp"], f32)
    b_sp = np.asarray(inputs["b_sp"], f32)
    W1 = np.asarray(inputs["W1"], f32)
    g1 = np.asarray(inputs["g1"], f32)
    be1 = np.asarray(inputs["be1"], f32)
    W2 = np.asarray(inputs["W2"], f32)
    g2 = np.asarray(inputs["g2"], f32)
    be2 = np.asarray(inputs["be2"], f32)

    # obs: (S, P, 16) with feature index t*2+c
    obs = np.transpose(traj[:OBS], (1, 0, 2)).reshape(S, P, OBS * 2)
    h = h_states.reshape(S, P, HDIM)

    # relT[s, r, i*64+j] = obs[s, j, r] - obs[s, i, r]
    obsT = obs.transpose(0, 2, 1)                      # (S, 16, P)
    relT = (obsT[:, :, None, :] - obsT[:, :, :, None]).reshape(S, 16, PP)

    # C fold: q(f) = (f//64)*2 + f%2
    f_idx = np.arange(EMB * OBS)
    qof = (f_idx // EMB) * 2 + (f_idx % 2)
    W1a, W1b = W1[:D1], W1[D1:]
    C = np.zeros((256, D1), f32)
    Cb = np.zeros((16, D1), f32)
    for q in range(16):
        m = qof == q
        C[q * 16:(q + 1) * 16] = W_sp[:, m] @ W1a[m]
        Cb[q] = b_sp[m] @ W1a[m]
    has_cb = bool(np.any(Cb != 0.0))
    Csb = np.ascontiguousarray(C.reshape(2, 128, D1).transpose(1, 0, 2))
    W2sb = np.ascontiguousarray(W2.reshape(4, 128, D2).transpose(1, 0, 2))

    shared = {
        "Csb": Csb.astype(dtm),
        "W1b": W1b.astype(dtm),
        "W2sb": W2sb.astype(dtm),
        "g1c": np.ascontiguousarray(g1.reshape(4, 128).T),
        "be1c": np.ascontiguousarray(be1.reshape(4, 128).T),
        "g2c": np.ascontiguousarray(g2.reshape(8, 128).T),
        "be2c": np.ascontiguousarray(be2.reshape(8, 128).T),
    }
    if has_cb:
        shared["Cb"] = Cb.astype(dtm)

    in_maps = []
    for c in range(NCORES):
        sl = slice(c * G, (c + 1) * G)
        reld = np.ascontiguousarray(relT[sl].reshape(G * 16, PP))
        twqT = np.ascontiguousarray(
            traj_weight[sl].transpose(3, 2, 0, 1).reshape(16, G, PP))
        hT = np.ascontiguousarray(h[sl].transpose(2, 0, 1))     # (64,G,P)
        in_maps.append({
            "reld": reld.astype(dtm),
            "twqT": twqT.astype(dtm),
            "hT": hT.astype(dtm),
            **shared,
        })
    return in_maps, has_cb


def kernel(**inputs) -> np.ndarray:
    global LAST_RESULTS
    from concourse import bass_utils

    in_maps, has_cb = _host_prepare(inputs)
    key = ("prog", has_cb)
    if key not in _PROG_CACHE:
        _PROG_CACHE[key] = build_program(has_cb=has_cb)
    nc = _PROG_CACHE[key]

    trace = bool(int(os.environ.get("KERNEL_TRACE", "0")))
    res = bass_utils.run_bass_kernel_spmd(
        nc, in_maps, core_ids=list(range(NCORES)), trace=trace)
    LAST_RESULTS = res
    out = np.concatenate([res.results[c]["out"] for c in range(NCORES)], axis=0)
    return out.astype(np.float32)
